# revision 2
# baseline (speedup 1.0000x reference)
"""Llama4 MoE experts kernel for 8 Trainium2 NeuronCores.

Expert-parallel: tokens are pre-sorted per expert (8192 tokens = 8 experts
x 1024 tokens), so core e gets expert e's tokens + weights and computes
   out_e = (up_e * silu(gate_e)) @ W2_e,   [gate_e|up_e] = x_e @ W1_e
entirely locally (no collectives). Matmuls run in bf16 with fp32 PSUM
accumulation; weights/activations are cast + laid out host-side so every
DMA is a long per-partition contiguous run and the PE streams at 1 row/cyc.

The first W1 tile and the token-front-half of x are DMA'd in fine-grained,
first-use order so the PE can start ~3us earlier; down-proj uses 512-wide
output blocks (one full PSUM bank) to halve phase-2 instruction count, and
outputs are stored as bf16 to shrink the drain tail.
"""

import numpy as np
import ml_dtypes

E, T, H, F, P = 8, 1024, 2048, 4096, 128
KH, KF = H // P, F // P          # 16 k-blocks over H, 32 over F
CB = (2 * F) // P                # 64 column blocks of W1 (gate 0..31, up 32..63)
HB = H // 512                    # 4 output-column blocks of 512
_CACHE = {}


def _build():
    import concourse.bacc as bacc
    import concourse.tile as tile
    import concourse.mybir as mybir

    bf16 = mybir.dt.bfloat16
    f32 = mybir.dt.float32

    nc = bacc.Bacc("TRN2", target_bir_lowering=False, debug=False, num_devices=E)

    xt_d = nc.dram_tensor("xt", [P, KH, T], bf16, kind="ExternalInput").ap()
    w1_d = nc.dram_tensor("w1", [CB, P, KH, P], bf16, kind="ExternalInput").ap()
    w2_d = nc.dram_tensor("w2", [HB, P, KF, 512], bf16, kind="ExternalInput").ap()
    out_d = nc.dram_tensor("out", [T, H], bf16, kind="ExternalOutput").ap()

    with tile.TileContext(nc) as tc:
        with (
            tc.tile_pool(name="resident", bufs=1) as res_pool,
            tc.tile_pool(name="w1pool", bufs=3) as w1_pool,
            tc.tile_pool(name="w2pool", bufs=2) as w2_pool,
            tc.tile_pool(name="tmppool", bufs=3) as tmp_pool,
            tc.tile_pool(name="outpool", bufs=4) as out_pool,
            tc.tile_pool(name="psg", bufs=2, space="PSUM") as psg_pool,
            tc.tile_pool(name="psu", bufs=2, space="PSUM") as psu_pool,
            tc.tile_pool(name="pso", bufs=4, space="PSUM") as pso_pool,
        ):
            xT = res_pool.tile([P, KH, T], bf16, name="xT")
            interT = res_pool.tile([P, KF, T], bf16, name="interT")

            # Phase 1: gate/up projections + SwiGLU -> interT (F on partitions)
            # i=0 DMAs are issued fine-grained in first-use order (w1g chunk,
            # then the x k-blocks that chunk's matmuls consume, front token
            # half first) so the first chain starts as soon as ~256KiB lands
            # instead of waiting for whole tiles.
            for i in range(KF):
                w1g = w1_pool.tile([P, KH, P], bf16, tag="w1g", name=f"w1g_{i}")
                w1u = w1_pool.tile([P, KH, P], bf16, tag="w1u", name=f"w1u_{i}")
                if i == 0:
                    for c in range(4):
                        ks = slice(4 * c, 4 * c + 4)
                        nc.sync.dma_start(out=w1g[:, ks, :], in_=w1_d[0, :, ks, :])
                        for kb in (2 * c, 2 * c + 1):
                            nc.sync.dma_start(
                                out=xT[:, kb, 0:512], in_=xt_d[:, kb, 0:512]
                            )
                    for c in range(4):
                        ks = slice(4 * c, 4 * c + 4)
                        nc.sync.dma_start(out=w1u[:, ks, :], in_=w1_d[KF, :, ks, :])
                        for kb in (8 + 2 * c, 8 + 2 * c + 1):
                            nc.sync.dma_start(
                                out=xT[:, kb, 0:512], in_=xt_d[:, kb, 0:512]
                            )
                    for c in range(4):
                        ks = slice(4 * c, 4 * c + 4)
                        nc.sync.dma_start(
                            out=xT[:, ks, 512:1024], in_=xt_d[:, ks, 512:1024]
                        )
                else:
                    nc.sync.dma_start(out=w1g[:], in_=w1_d[i])
                    nc.sync.dma_start(out=w1u[:], in_=w1_d[KF + i])
                for th in range(2):
                    ts_ = slice(th * 512, (th + 1) * 512)
                    pg = psg_pool.tile([P, 512], f32, tag="pg", name=f"pg_{i}_{th}")
                    pu = psu_pool.tile([P, 512], f32, tag="pu", name=f"pu_{i}_{th}")
                    for kb in range(KH):
                        nc.tensor.matmul(
                            pg[:], lhsT=w1g[:, kb, :], rhs=xT[:, kb, ts_],
                            start=(kb == 0), stop=(kb == KH - 1),
                        )
                    for kb in range(KH):
                        nc.tensor.matmul(
                            pu[:], lhsT=w1u[:, kb, :], rhs=xT[:, kb, ts_],
                            start=(kb == 0), stop=(kb == KH - 1),
                        )
                    sg = tmp_pool.tile([P, 512], f32, tag="sg", name=f"sg_{i}_{th}")
                    nc.scalar.activation(
                        sg[:], pg[:], mybir.ActivationFunctionType.Silu
                    )
                    nc.vector.tensor_mul(interT[:, i, ts_], sg[:], pu[:])

            # Phase 2: down projection, streaming W2 once; 512-wide output
            # blocks fill a whole PSUM bank per matmul (half the instructions
            # of 256-wide blocks).
            for hb in range(HB):
                w2t = w2_pool.tile([P, KF, 512], bf16, tag="w2", name=f"w2_{hb}")
                nc.sync.dma_start(out=w2t[:], in_=w2_d[hb])
                for tb in range(T // P):
                    po = pso_pool.tile([P, 512], f32, tag="po", name=f"po_{hb}_{tb}")
                    for kb in range(KF):
                        nc.tensor.matmul(
                            po[:],
                            lhsT=interT[:, kb, tb * P:(tb + 1) * P],
                            rhs=w2t[:, kb, :],
                            start=(kb == 0), stop=(kb == KF - 1),
                        )
                    ob = out_pool.tile([P, 512], bf16, tag="ob", name=f"ob_{hb}_{tb}")
                    nc.scalar.copy(ob[:], po[:])
                    nc.sync.dma_start(
                        out=out_d[tb * P:(tb + 1) * P, hb * 512:(hb + 1) * 512],
                        in_=ob[:],
                    )

    nc.compile()
    return nc


def _prep_inputs(hidden_states, gate_up_proj, down_proj):
    bf = ml_dtypes.bfloat16
    xr = np.asarray(hidden_states, np.float32).reshape(E, T, H)
    # xt[e, p, k, t] = x[e, t, k*128+p]
    xt = xr.transpose(0, 2, 1).reshape(E, KH, P, T).transpose(0, 2, 1, 3)
    xt = np.ascontiguousarray(xt).astype(bf)
    # w1b[e, c, p, k, j] = W1[e, k*128+p, c*128+j]
    w1b = np.asarray(gate_up_proj, np.float32).reshape(E, KH, P, CB, P)
    w1b = np.ascontiguousarray(w1b.transpose(0, 3, 2, 1, 4)).astype(bf)
    # w2b[e, hb, p, kb, j] = W2[e, kb*128+p, hb*512+j]
    w2b = np.asarray(down_proj, np.float32).reshape(E, KF, P, HB, 512)
    w2b = np.ascontiguousarray(w2b.transpose(0, 3, 2, 1, 4)).astype(bf)
    return [
        {"xt": np.ascontiguousarray(xt[e]),
         "w1": np.ascontiguousarray(w1b[e]),
         "w2": np.ascontiguousarray(w2b[e])}
        for e in range(E)
    ]


def run_spmd(in_maps, trace=False, trace_kwargs=None):
    from concourse.bass_utils import run_bass_kernel_spmd
    from concourse.bass_interp import get_hw_module

    if "nc" not in _CACHE:
        _CACHE["nc"] = _build()
    nc = _CACHE["nc"]

    old_m = nc.m
    nc.m = get_hw_module(nc.m)
    try:
        res = run_bass_kernel_spmd(
            nc, in_maps, core_ids=list(range(E)),
            trace=trace, **(trace_kwargs or {}),
        )
    finally:
        nc.m = old_m
    return res


def kernel(hidden_states, gate_up_proj, down_proj):
    in_maps = _prep_inputs(hidden_states, gate_up_proj, down_proj)
    res = run_spmd(in_maps)
    out = np.concatenate(
        [np.asarray(res.results[e]["out"]) for e in range(E)], axis=0
    )
    return out.astype(np.float32)
